# revision 12
# baseline (speedup 1.0000x reference)
"""DeFUM dense-transformer kernel for 8x Trainium2 NeuronCores.

Sharding: data-parallel over batch B=64 -> 8 batch elements per core.
Each core runs the full network (semantic attention + 4 encoder layers)
on its shard; no collectives.

v2 design (vs v1 baseline at 1871us):
- fp8e4m3 DoubleRow matmuls for all weight GEMMs (qkv / out-proj / ffn):
  weights host-scaled by 2^11, activations device-scaled by 2^5; scales
  are powers of two folded into existing eviction ops (exact).
- All activation transposes moved off the PE onto the DMA XBAR
  (dma_start_transpose on the SP queue); h's bf16 "shadow" (64x LN
  output) is written by gpsimd at LN time and transposed from there.
- Attention core (scores/sums/av, K=64/M=64) stays bf16 but issues
  row/col tile-mates adjacently so pairs run concurrently on the PE;
  softmax normalization is one gpsimd divide (ones-matmul computes
  64x-denominators broadcast across partitions).
- GEMMs process batch PAIRS (512-token moving operands) so DoubleRow's
  256-column LDWEIGHTS is amortized.

Residual h stays f32 token-major [128, 16, 768]; LayerNorm is token-major
(bn_stats on DVE). PSUM: 2 mm bufs + 4 score bufs + pav + sums = 8 banks.
"""

import numpy as np
import ml_dtypes

import concourse.bass as bass
import concourse.mybir as mybir
import concourse.tile as tile
from concourse.bass_utils import run_bass_kernel_spmd

B, N_OCR, N_OBJ, D, H, L, DFF = 64, 64, 192, 768, 12, 4, 2048
N = N_OCR + N_OBJ            # 256 tokens per batch element
DH = D // H                  # 64
NCORES = 8
BL = B // NCORES             # 8 batch elements per core
T = BL * N                   # 2048 tokens per core
TC = T // 128                # 16 token chunks
DC = D // 128                # 6 d chunks
KC = DC // 2                 # 3 DoubleRow k-chunks over D
FC = DFF // 128              # 16 dff chunks
FKC = FC // 2                # 8 DoubleRow k-chunks over DFF
NP = BL // 2                 # 4 batch pairs
EPS = 1e-5
AF = mybir.ActivationFunctionType
ALU = mybir.AluOpType
DR = mybir.MatmulPerfMode.DoubleRow
F32 = mybir.dt.float32
BF16 = mybir.dt.bfloat16
FP8 = mybir.dt.float8e4

EXP_W = 11                   # weights scaled by 2^11 on host
EXP_A = 5                    # activations scaled by 2^5 on device
S_W = float(2.0 ** EXP_W)
S_A = float(2.0 ** EXP_A)
SCL_QK = 2.0 ** (-EXP_W)             # psum -> qkT (leaves 2^5 * natural)
SCL_NAT = 2.0 ** (-EXP_W - EXP_A)    # psum -> natural
SCL_A = 2.0 ** (-EXP_A)              # 2^5-scaled bf16 psum -> natural
LN64 = float(np.log(S_A))


def _build(flags):
    nc = bass.Bass()

    x_d = nc.declare_dram_parameter("x", [T, D], F32, isOutput=False)
    xT8_d = nc.declare_dram_parameter("xT8", [D, T], FP8, isOutput=False)
    dvr_d = nc.declare_dram_parameter("dv_rows", [128, BL, N], BF16, isOutput=False)
    dvc_d = nc.declare_dram_parameter("dv_cols", [128, 2 * BL], F32, isOutput=False)
    wqkvT_d = nc.declare_dram_parameter("wqkvT8", [L + 1, D, 3 * D], FP8, False)
    woT_d = nc.declare_dram_parameter("woT8", [L, D, D], FP8, False)
    w1T_d = nc.declare_dram_parameter("w1T", [L, D, DFF], BF16, False)
    w2T_d = nc.declare_dram_parameter("w2T", [L, DFF, D], BF16, False)
    qkb_d = nc.declare_dram_parameter("qkb_cols", [L + 1, 128, 12], F32, False)
    vb_d = nc.declare_dram_parameter("vb_cols", [L + 1, 128, 6], F32, False)
    w1b_d = nc.declare_dram_parameter("w1b_cols", [L, 128, FC], F32, False)
    brow_d = nc.declare_dram_parameter("bias_rows", [1 + 2 * L, D], F32, False)
    lng_d = nc.declare_dram_parameter("ln_g", [1 + 2 * L, D], F32, False)
    lnb_d = nc.declare_dram_parameter("ln_b", [1 + 2 * L, D], F32, False)
    out_d = nc.declare_dram_parameter("out", [BL, N_OCR, D], F32, isOutput=True)

    def bcast_ap(src_ap, parts=128):
        return bass.AP(
            tensor=src_ap.tensor,
            offset=src_ap.offset,
            ap=[[0, parts]] + list(src_ap.ap),
        )

    with tile.TileContext(nc) as tc_:
        _emit(nc, tc_, flags, locals())
    _fix_waits(nc)
    return nc


def _fix_waits(nc):
    """Single-wait-slot toolchain: hoist extra waits onto same-engine NOPs.
    DmaTranspose (XPOSE) instructions accept NO wait -> hoist all of them."""
    import bass_rust

    n_new = 0
    for bb in nc.main_func.blocks:
        out = []
        changed = False
        for inst in bb.instructions:
            limit = 0 if "DmaTranspose" in type(inst).__name__ else 1
            si = inst.sync_info
            waits = list(si.on_wait) if si is not None and si.on_wait else []
            if len(waits) > limit:
                changed = True
                keep = waits[-limit:] if limit else []
                for w in waits[: len(waits) - limit]:
                    nop = bass_rust.InstNoOp(name=f"I-wsplit-{n_new}")
                    n_new += 1
                    nop.engine = inst.engine
                    nop.sync_info = bass_rust.SyncInfo(on_wait=[w], on_update=[])
                    nc.register_instruction(nop)
                    out.append(nop)
                inst.sync_info = bass_rust.SyncInfo(
                    on_wait=keep,
                    on_update=list(si.on_update) if si.on_update else [],
                )
            out.append(inst)
        if changed:
            bb.instructions = out
    return n_new


def _emit(nc, tc, flags, d):
    x_d, xT8_d, dvr_d, dvc_d = d["x_d"], d["xT8_d"], d["dvr_d"], d["dvc_d"]
    wqkvT_d, woT_d, w1T_d, w2T_d = d["wqkvT_d"], d["woT_d"], d["w1T_d"], d["w2T_d"]
    qkb_d, vb_d, w1b_d = d["qkb_d"], d["vb_d"], d["w1b_d"]
    brow_d, lng_d, lnb_d, out_d = d["brow_d"], d["lng_d"], d["lnb_d"], d["out_d"]
    bcast_ap = d["bcast_ap"]

    from contextlib import ExitStack
    ctx = ExitStack()
    const = ctx.enter_context(tc.tile_pool(name="const", bufs=1))
    wpool = ctx.enter_context(tc.tile_pool(name="w", bufs=1))
    act = ctx.enter_context(tc.tile_pool(name="act", bufs=1))
    tpool = ctx.enter_context(tc.tile_pool(name="tp", bufs=2))   # hT/hT8 pair tiles
    apool = ctx.enter_context(tc.tile_pool(name="ap", bufs=1))   # qkT/v/aoT/gT
    wk = ctx.enter_context(tc.tile_pool(name="wk", bufs=2))      # small scratch
    ps_mm = ctx.enter_context(tc.tile_pool(name="psm", bufs=2, space="PSUM"))
    ps_sc = ctx.enter_context(tc.tile_pool(name="pssc", bufs=6, space="PSUM"))

    # ---- constants ----
    ones_bf = const.tile([128, 64], BF16)
    nc.vector.memset(ones_bf, float(2.0 ** (-EXP_A)))   # encoder sums -> 2^-5 * sum
    ones8 = const.tile([128, 2, 16], FP8)
    nc.vector.memset(ones8, S_A)                         # sem sums -> 2^5 * sum
    eps_t = const.tile([128, 1], F32)
    nc.vector.memset(eps_t, EPS)
    qkb_sb = vb_sb = w1b_sb = None
    if flags["qkb"]:
        qkb_sb = const.tile([128, L + 1, 12], F32)
        nc.sync.dma_start(qkb_sb, qkb_d[:].rearrange("l p c -> p l c"))
    if flags["vb"]:
        vb_sb = const.tile([128, L + 1, 6], F32)
        nc.sync.dma_start(vb_sb, vb_d[:].rearrange("l p c -> p l c"))
    if flags["w1b"]:
        w1b_sb = const.tile([128, L, FC], F32)
        nc.sync.dma_start(w1b_sb, w1b_d[:].rearrange("l p c -> p l c"))
    ldvc = const.tile([128, 2 * BL], F32)
    nc.sync.dma_start(ldvc, dvc_d[:])
    nc.scalar.activation(ldvc, ldvc, AF.Ln)
    ldvr_raw = const.tile([128, BL, N], BF16)
    nc.sync.dma_start(ldvr_raw, dvr_d[:])
    ldvr_all = const.tile([128, BL, N], F32)
    nc.scalar.activation(ldvr_all, ldvr_raw, AF.Ln)

    # residual stream + its 2^5-scaled bf16 shadow (LN outputs)
    h = act.tile([128, TC, D], F32)
    nc.sync.dma_start(h, x_d[:].rearrange("(c p) d -> p c d", p=128))
    shadow = act.tile([128, TC, D], BF16)

    def ln_chunk(c, g_bc=None, b_bc=None, want_shadow=True):
        """In-place LayerNorm of h[:, c, :]; also writes shadow = 2^5*h (bf16)."""
        stats = wk.tile([128, 2, 6], F32, tag="bnst")
        nc.vector.bn_stats(stats[:, 0, :], h[:, c, 0:384])
        nc.vector.bn_stats(stats[:, 1, :], h[:, c, 384:768])
        mv = wk.tile([128, 2], F32, tag="bnmv")
        nc.vector.bn_aggr(mv, stats)
        rs = wk.tile([128, 1], F32, tag="bnrs")
        nc.scalar.activation(rs, mv[:, 1:2], AF.Ln, bias=eps_t)
        nc.scalar.activation(rs, rs, AF.Exp, scale=-0.5)
        nc.vector.tensor_scalar(
            out=h[:, c, :], in0=h[:, c, :],
            scalar1=mv[:, 0:1], scalar2=rs,
            op0=ALU.subtract, op1=ALU.mult,
        )
        if g_bc is not None:
            nc.vector.tensor_mul(h[:, c, :], h[:, c, :], g_bc)
        if b_bc is not None:
            nc.vector.tensor_add(h[:, c, :], h[:, c, :], b_bc)
        if want_shadow:
            nc.gpsimd.tensor_scalar_mul(shadow[:, c, :], h[:, c, :], S_A)

    def load_brow(tag, src_ap):
        t = wk.tile([128, D], F32, tag=tag)
        nc.gpsimd.dma_start(t, bcast_ap(src_ap))
        return t

    def qkv_pair(p, wqkv8, hT8, li, v_out, v_scale):
        """q,k -> qkT (bf16, 2^5*nat); v -> v_out (v_scale applied)."""
        qkT = apool.tile([128, 12, 512], FP8, tag="qkT")
        for oc in range(12):
            pq = ps_mm.tile([128, 512], F32, tag="mm")
            for kc in range(KC):
                nc.tensor.matmul(pq, wqkv8[:, 2 * kc:2 * kc + 2, 128 * oc:128 * (oc + 1)],
                                 hT8[:, 2 * kc:2 * kc + 2, :], start=(kc == 0),
                                 stop=(kc == KC - 1), perf_mode=DR)
            if flags["qkb"]:
                nc.vector.tensor_scalar(
                    out=qkT[:, oc, :], in0=pq, scalar1=SCL_QK,
                    scalar2=qkb_sb[:, li, oc:oc + 1], op0=ALU.mult, op1=ALU.add)
            else:
                nc.vector.tensor_scalar_mul(qkT[:, oc, :], pq, SCL_QK)
        for tc4 in range(4):
            for ev, (e0, e1) in enumerate(((0, 512), (512, 768))):
                pv = ps_mm.tile([128, 512], F32, tag="mm")
                for kc in range(KC):
                    nc.tensor.matmul(pv[:, :e1 - e0],
                                     hT8[:, 2 * kc:2 * kc + 2, 128 * tc4:128 * (tc4 + 1)],
                                     wqkv8[:, 2 * kc:2 * kc + 2, 2 * D + e0:2 * D + e1],
                                     start=(kc == 0), stop=(kc == KC - 1), perf_mode=DR)
                nc.vector.tensor_scalar_mul(v_out[:, tc4, e0:e1], pv[:, :e1 - e0],
                                            v_scale)
        return qkT

    # ================= semantic attention =================
    wqkv8 = wpool.tile([128, DC, 3 * D], FP8, tag="wqkv")
    nc.scalar.dma_start(wqkv8, wqkvT_d[0].rearrange("(c p) o -> p c o", p=128))

    sem_vb_bc = load_brow("sem_vb", brow_d[0]) if flags["sem_vb"] else None
    ln0_g_bc = load_brow("lng", lng_d[0]) if flags["lngb"] else None
    ln0_b_bc = load_brow("lnb", lnb_d[0]) if flags["lngb"] else None

    sem_scale = float(2.0 ** (-2 * EXP_A) / np.sqrt(np.float32(D)))
    for p in range(NP):
        hT8 = tpool.tile([128, DC, 512], FP8, tag="hT8")
        nc.sync.dma_start(
            hT8, xT8_d[:].rearrange("(c p) t -> p c t", p=128)[:, :, 512 * p:512 * (p + 1)])
        v8 = apool.tile([128, 4, D], FP8, tag="v")
        qkT = qkv_pair(p, wqkv8, hT8, 0, v8, SCL_QK)
        for b2 in range(2):
            b = 2 * p + b2
            cb = 256 * b2
            # R bias
            R = wk.tile([128, 2, N], F32, tag="R", bufs=1)
            for ic in range(2):
                nc.vector.tensor_scalar(
                    out=R[:, ic, :], in0=ldvr_all[:, b, :],
                    scalar1=ldvc[:, 2 * b + ic:2 * b + ic + 1],
                    scalar2=None, op0=ALU.subtract)
            nc.scalar.activation(R, R, AF.Abs)
            # scores (bf16, K=128 x 6 chunks)
            psc = ps_sc.tile([128, 2, N], F32, tag="sc")
            for jc in range(2):
                for dc in range(DC):
                    nc.tensor.matmul(psc[:, jc, :],
                                     qkT[:, 6 + dc, cb + 128 * jc:cb + 128 * (jc + 1)],
                                     qkT[:, dc, cb:cb + N],
                                     start=(dc == 0), stop=(dc == DC - 1))
            nc.vector.scalar_tensor_tensor(
                out=R, in0=psc, scalar=sem_scale, in1=R, op0=ALU.mult, op1=ALU.add)
            expT8 = wk.tile([128, 2, N], FP8, tag="expT8", bufs=1)
            nc.scalar.activation(expT8, R, AF.Exp)
            # denominators: 2^5 * sum  [i-part, ic]
            ps_cs = ps_sc.tile([128, 2], F32, tag="sc")
            for ic in range(2):
                nc.tensor.matmul(ps_cs[:, ic:ic + 1],
                                 expT8[:, :, 128 * ic:128 * (ic + 1)],
                                 ones8[:, :, 0:1], start=True, stop=True, perf_mode=DR)
            rec2 = wk.tile([128, 2], F32, tag="rec2")
            nc.vector.reciprocal(rec2, ps_cs)
            # av (2^5*prenorm) + normalize + residual into h
            for ic in range(2):
                c = 2 * b + ic
                for ev, (e0, e1) in enumerate(((0, 384), (384, 768))):
                    pa = ps_mm.tile([128, 512], F32, tag="mm")
                    nc.tensor.matmul(pa[:, :384],
                                     expT8[:, :, 128 * ic:128 * (ic + 1)],
                                     v8[:, 2 * b2:2 * b2 + 2, e0:e1],
                                     start=True, stop=True, perf_mode=DR)
                    nc.vector.scalar_tensor_tensor(
                        out=h[:, c, e0:e1], in0=pa[:, :384],
                        scalar=rec2[:, ic:ic + 1], in1=h[:, c, e0:e1],
                        op0=ALU.mult, op1=ALU.add)
                    if sem_vb_bc is not None:
                        nc.vector.tensor_add(h[:, c, e0:e1], h[:, c, e0:e1],
                                             sem_vb_bc[:, e0:e1])
                ln_chunk(c, ln0_g_bc, ln0_b_bc)

    # ================= encoder layers =================
    enc_scale = float(2.0 ** (-2 * EXP_A) / np.sqrt(np.float32(DH)))
    for li in range(L):
        wqkv8 = wpool.tile([128, DC, 3 * D], FP8, tag="wqkv")
        nc.scalar.dma_start(wqkv8, wqkvT_d[li + 1].rearrange("(c p) o -> p c o", p=128))
        wo8 = wpool.tile([128, DC, D], FP8, tag="wo")
        nc.scalar.dma_start(wo8, woT_d[li].rearrange("(c p) o -> p c o", p=128))
        w1 = wpool.tile([128, DC, DFF], BF16, tag="w1")
        nc.scalar.dma_start(w1, w1T_d[li].rearrange("(c p) o -> p c o", p=128))
        w2 = wpool.tile([128, FC, D], BF16, tag="w2")
        nc.scalar.dma_start(w2, w2T_d[li].rearrange("(c p) o -> p c o", p=128))

        wob_bc = load_brow("wob", brow_d[1 + 2 * li]) if flags["wob"] else None
        w2b_bc = load_brow("w2b", brow_d[2 + 2 * li]) if flags["w2b"] else None
        g1_bc = load_brow("lng", lng_d[1 + 2 * li]) if flags["lngb"] else None
        b1_bc = load_brow("lnb", lnb_d[1 + 2 * li]) if flags["lngb"] else None
        g2_bc = load_brow("lng2", lng_d[2 + 2 * li]) if flags["lngb"] else None
        b2_bc = load_brow("lnb2", lnb_d[2 + 2 * li]) if flags["lngb"] else None

        # ---- attention, per batch pair ----
        for p in range(NP):
            hT = tpool.tile([128, DC, 512], BF16, tag="hT")
            for t4 in range(4):
                nc.sync.dma_start_transpose(hT[:, :, 128 * t4:128 * (t4 + 1)],
                                            shadow[:, 4 * p + t4, :])
            hT8 = tpool.tile([128, DC, 512], FP8, tag="hT8")
            nc.gpsimd.tensor_copy(hT8, hT)
            v_p = apool.tile([128, 4, D], BF16, tag="v")
            qkT = qkv_pair(p, wqkv8, hT8, li + 1, v_p, SCL_NAT)
            aoT8 = apool.tile([128, DC, 512], FP8, tag="aoT")
            # attention core, group = (b2, ec2) = 4 heads. Per group: 4 score
            # psum tiles + pav + sums cycle through the 6-buf ps_sc pool; the
            # two-pass emission (all scores, then sums+av) hides exp latency,
            # and (sub0, sub1) tile-mates issue adjacently so the K=64 / M=64
            # matmuls run concurrently on different row/col groups.
            for b2 in range(2):
                cb = 256 * b2
                for ec2 in range(3):
                    scs = []
                    exps = []
                    for ecs in range(2):
                        ec = 2 * ec2 + ecs
                        sc2 = [ps_sc.tile([128, 2, N], F32, tag="sc", name=f"sc{s}")
                               for s in range(2)]
                        for jc in range(2):
                            for sub in range(2):
                                off = 64 * sub
                                nc.tensor.matmul(
                                    sc2[sub][:, jc, :],
                                    qkT[off:off + 64, 6 + ec, cb + 128 * jc:cb + 128 * (jc + 1)],
                                    qkT[off:off + 64, ec, cb:cb + N],
                                    start=True, stop=True)
                        ex2 = []
                        for sub in range(2):
                            e_t = wk.tile([128, 2, N], BF16, tag=f"expT{ecs}{sub}")
                            nc.scalar.activation(e_t, sc2[sub], AF.Exp, scale=enc_scale)
                            ex2.append(e_t)
                        scs.append(sc2)
                        exps.append(ex2)
                    pav = ps_sc.tile([128, 2, N], F32, tag="sc")
                    ps_sums = ps_sc.tile([128, 2, N], F32, tag="sc")
                    for ecs in range(2):
                        ec = 2 * ec2 + ecs
                        for jc in range(2):
                            for sub in range(2):
                                off = 64 * sub
                                nc.tensor.matmul(
                                    ps_sums[off:off + 64, ecs, :],
                                    ones_bf[:, 0:64], exps[ecs][sub][:, jc, :],
                                    start=(jc == 0), stop=(jc == 1),
                                    skip_group_check=True)
                        for jc in range(2):
                            for sub in range(2):
                                off = 64 * sub
                                hd = 2 * ec + sub
                                nc.tensor.matmul(
                                    pav[off:off + 64, ecs, :],
                                    v_p[:, 2 * b2 + jc, DH * hd:DH * (hd + 1)],
                                    exps[ecs][sub][:, jc, :],
                                    start=(jc == 0), stop=(jc == 1),
                                    skip_group_check=True)
                    # rec = 1/(2^-5 * sums) on ACT (shares the Exp table);
                    # aoT = pav * rec -> 2^5 * attn_out, fp8
                    rec = wk.tile([128, 2, N], F32, tag="rec")
                    nc.scalar.activation(rec, ps_sums, AF.Ln)
                    nc.scalar.activation(rec, rec, AF.Exp, scale=-1.0)
                    nc.vector.tensor_tensor(
                        out=aoT8[:, 2 * ec2:2 * ec2 + 2, cb:cb + N],
                        in0=pav, in1=rec, op=ALU.mult)
                    if flags["vb"]:
                        for ecs in range(2):
                            nc.gpsimd.tensor_scalar_add(
                                aoT8[:, 2 * ec2 + ecs, cb:cb + N],
                                aoT8[:, 2 * ec2 + ecs, cb:cb + N],
                                vb_sb[:, li + 1, 2 * ec2 + ecs:2 * ec2 + ecs + 1])
            # out-projection + residual + LN1
            for tc4 in range(4):
                c = 4 * p + tc4
                for fh in range(2):
                    po = ps_mm.tile([128, 512], F32, tag="mm")
                    for kc in range(KC):
                        nc.tensor.matmul(
                            po[:, :384],
                            aoT8[:, 2 * kc:2 * kc + 2, 128 * tc4:128 * (tc4 + 1)],
                            wo8[:, 2 * kc:2 * kc + 2, 384 * fh:384 * (fh + 1)],
                            start=(kc == 0), stop=(kc == KC - 1), perf_mode=DR)
                    sl = slice(384 * fh, 384 * (fh + 1))
                    nc.vector.scalar_tensor_tensor(
                        out=h[:, c, sl], in0=po[:, :384], scalar=SCL_NAT,
                        in1=h[:, c, sl], op0=ALU.mult, op1=ALU.add)
                    if wob_bc is not None:
                        nc.vector.tensor_add(h[:, c, sl], h[:, c, sl], wob_bc[:, sl])
                ln_chunk(c, g1_bc, b1_bc)

        # ---- FFN over 256-token chunks (bf16; fp8 here costs accuracy) ----
        for nt in range(8):
            h1T = tpool.tile([128, DC, 256], BF16, tag="hT")
            for t2 in range(2):
                nc.sync.dma_start_transpose(h1T[:, :, 128 * t2:128 * (t2 + 1)],
                                            shadow[:, 2 * nt + t2, :])
            gT = apool.tile([128, FC, 256], BF16, tag="gT")
            for fc in range(FC):
                pf = ps_mm.tile([128, 512], F32, tag="mm")
                for dc in range(DC):
                    nc.tensor.matmul(pf[:, :256],
                                     w1[:, dc, 128 * fc:128 * (fc + 1)],
                                     h1T[:, dc, :],
                                     start=(dc == 0), stop=(dc == DC - 1))
                if flags["w1b"]:
                    nc.scalar.activation(gT[:, fc, :], pf[:, :256], AF.Gelu,
                                         scale=SCL_A, bias=w1b_sb[:, li, fc:fc + 1])
                else:
                    nc.scalar.activation(gT[:, fc, :], pf[:, :256], AF.Gelu,
                                         scale=SCL_A)
            for m4 in range(2):
                c = 2 * nt + m4
                for eh, (e0, e1) in enumerate(((0, 512), (512, 768))):
                    pf2 = ps_mm.tile([128, 512], F32, tag="mm")
                    for fc in range(FC):
                        nc.tensor.matmul(pf2[:, :e1 - e0],
                                         gT[:, fc, 128 * m4:128 * (m4 + 1)],
                                         w2[:, fc, e0:e1],
                                         start=(fc == 0), stop=(fc == FC - 1))
                    nc.vector.tensor_add(h[:, c, e0:e1], pf2[:, :e1 - e0],
                                         h[:, c, e0:e1])
                    if w2b_bc is not None:
                        nc.vector.tensor_add(h[:, c, e0:e1], h[:, c, e0:e1],
                                             w2b_bc[:, e0:e1])
                ln_chunk(c, g2_bc, b2_bc, want_shadow=(li < L - 1))

    # ---- output: first 64 tokens (OCR) of each batch element ----
    for b in range(BL):
        nc.sync.dma_start(out_d[b], h[0:64, 2 * b, :])
    ctx.close()


def _stage(inputs):
    """Host-side staging: shard + pre-layout + fp8 weight quantization."""
    f32 = np.float32
    fp8 = ml_dtypes.float8_e4m3

    def to8(a, scale):
        return np.clip(np.asarray(a, f32) * scale, -448.0, 448.0).astype(fp8)

    ocr = np.asarray(inputs["ocr_feats"], f32)
    obj = np.asarray(inputs["obj_feats"], f32)
    dv = np.concatenate([np.asarray(inputs["ocr_dvs"], f32),
                         np.asarray(inputs["obj_dvs"], f32)], axis=1)[..., 0]
    x = np.concatenate([ocr, obj], axis=1)  # [B, N, D]

    sem_qkv = np.concatenate([np.asarray(inputs["sa_wq"], f32),
                              np.asarray(inputs["sa_wk"], f32),
                              np.asarray(inputs["sa_wv"], f32)], axis=0)  # [3D, D]
    qkv_w = np.asarray(inputs["qkv_w"], f32)
    wqkvT = np.stack([sem_qkv.T] + [qkv_w[l].T for l in range(L)])  # [5, D, 3D]
    woT = np.stack([np.asarray(inputs["out_w"], f32)[l].T for l in range(L)])
    w1T = np.stack([np.asarray(inputs["ff1_w"], f32)[l].T for l in range(L)])
    w2T = np.stack([np.asarray(inputs["ff2_w"], f32)[l].T for l in range(L)])

    sem_b = np.concatenate([np.asarray(inputs["sa_bq"], f32),
                            np.asarray(inputs["sa_bk"], f32),
                            np.asarray(inputs["sa_bv"], f32)])
    qkvb = np.concatenate([sem_b[None], np.asarray(inputs["qkv_b"], f32)])  # [5,3D]
    # q/k biases land on qkT which carries 2^5 scale
    qkb_cols = (qkvb[:, :2 * D] * (2.0 ** EXP_A)).reshape(L + 1, 12, 128) \
        .transpose(0, 2, 1).copy()
    # v biases land on aoT8 (2^5 scale)
    vb_cols = (qkvb[:, 2 * D:] * (2.0 ** EXP_A)).reshape(L + 1, 6, 128) \
        .transpose(0, 2, 1).copy()
    w1b_cols = (np.asarray(inputs["ff1_b"], f32)
                .reshape(L, FC, 128).transpose(0, 2, 1).copy())

    out_b = np.asarray(inputs["out_b"], f32)
    ff2_b = np.asarray(inputs["ff2_b"], f32)
    sem_vb = sem_b[2 * D:]
    bias_rows = np.zeros((1 + 2 * L, D), f32)
    bias_rows[0] = sem_vb
    for l in range(L):
        bias_rows[1 + 2 * l] = out_b[l]
        bias_rows[2 + 2 * l] = ff2_b[l]
    ln_g = np.concatenate([np.asarray(inputs["ln0_g"], f32)[None],
                           np.stack([v for pair in zip(
                               np.asarray(inputs["ln1_g"], f32),
                               np.asarray(inputs["ln2_g"], f32)) for v in pair])])
    ln_b = np.concatenate([np.asarray(inputs["ln0_b"], f32)[None],
                           np.stack([v for pair in zip(
                               np.asarray(inputs["ln1_b"], f32),
                               np.asarray(inputs["ln2_b"], f32)) for v in pair])])

    flags = {
        "qkb": bool(np.any(qkvb[:, :2 * D] != 0)),
        "vb": bool(np.any(qkvb[1:, 2 * D:] != 0)),
        "sem_vb": bool(np.any(sem_vb != 0)),
        "wob": bool(np.any(out_b != 0)),
        "w2b": bool(np.any(ff2_b != 0)),
        "w1b": bool(np.any(np.asarray(inputs["ff1_b"], f32) != 0)),
        "lngb": bool(np.any(ln_g != 1) or np.any(ln_b != 0)),
    }

    shared = {
        "wqkvT8": to8(wqkvT, S_W),
        "woT8": to8(woT, S_W),
        "w1T": w1T.astype(ml_dtypes.bfloat16),
        "w2T": w2T.astype(ml_dtypes.bfloat16),
        "qkb_cols": qkb_cols, "vb_cols": vb_cols, "w1b_cols": w1b_cols,
        "bias_rows": bias_rows, "ln_g": ln_g, "ln_b": ln_b,
    }
    in_maps = []
    for c in range(NCORES):
        xs = x[c * BL:(c + 1) * BL].reshape(T, D)
        dvs = dv[c * BL:(c + 1) * BL]
        in_maps.append(dict(
            shared,
            x=np.ascontiguousarray(xs),
            xT8=to8(np.ascontiguousarray(xs.T), S_A),
            dv_rows=np.ascontiguousarray(
                np.broadcast_to(dvs[None], (128, BL, N))).astype(ml_dtypes.bfloat16),
            dv_cols=np.ascontiguousarray(
                dvs.reshape(BL, 2, 128).transpose(2, 0, 1).reshape(128, 2 * BL)),
        ))
    return in_maps, flags


_CACHE = {}


def _get_nc(flags):
    key = tuple(sorted(flags.items()))
    if key not in _CACHE:
        _CACHE[key] = _build(flags)
    return _CACHE[key]


def kernel(**inputs):
    in_maps, flags = _stage(inputs)
    nc = _get_nc(flags)
    res = run_bass_kernel_spmd(nc, in_maps, list(range(NCORES)))
    outs = [res.results[c]["out"] for c in range(NCORES)]
    return np.concatenate(outs, axis=0).astype(np.float32)
